# revision 49
# baseline (speedup 1.0000x reference)
"""Trainium2 Bass kernel for a GQA sliding-window attention layer.

Reference computation (B=2, T=2048, C=2048, 16 Q heads / 4 KV heads, d=128):
    q = x @ Wq; k = x @ Wk; v = x @ Wv (+ sigmoid-gated value embedding)
    q, k = rmsnorm(rope(q)), rmsnorm(rope(k))
    scores masked to the band 0 <= j - i < window (=1024), softmax over j
    out = (p @ v) @ Wo

Sharding: 8 cores = 2 batches x 4 KV groups.  Each core computes its 4 Q
heads / 1 KV head for one batch and a partial output (its 512-row slice of
the Wo contraction); the host sums the 4 partials per batch.

Layout strategy per core:
  - xT (C x T, bf16) resident in SBUF; all projections contract over C.
  - q̂T / k̂T kept [d=128 partitions, T free]; scores computed transposed
    (S^T tiles [kj, qi]) so that P^T feeds the PV matmul directly with v in
    natural [token, d] layout (no P transposes).
  - softmax has no max-subtraction: rms-normalized q,k bound |score| by
    sqrt(128), so exp is safe in fp32.
  - partition-broadcast reductions (rms rows, softmax denominators) are done
    in ONE cheap bf16 matmul with an all-ones [128,128] lhsT: every output
    row equals the partition-sum, so no separate row-extract + re-broadcast.
  - softmax denominators are pre-summed over k-tiles on the DVE (bf16) so
    the PE does a single reduction matmul per q-tile instead of nine.
"""

import numpy as np
import ml_dtypes
from collections import deque

BF16 = ml_dtypes.bfloat16

# Problem dims (hardcoded per contest rules)
B, T, C = 2, 2048, 2048
N_HEAD, N_KV, HD, GATE_CH = 16, 4, 128, 32
WINDOW = 1024
P = 128
GH = N_HEAD // N_KV  # q heads per kv head (= per core)
N_CORES = 8

_PROGRAM_CACHE = {}


def build_program(T_=T, C_=C, win=WINDOW):
    import concourse.mybir as mybir
    import concourse.tile as tile
    from concourse import bacc

    dt = mybir.dt
    f32 = dt.float32
    bf16 = dt.bfloat16
    AF = mybir.ActivationFunctionType
    ALU = mybir.AluOpType

    NT = T_ // P          # token tiles
    KT = C_ // P          # contraction tiles
    WT = win // P         # window tiles
    ISQ = 1.0 / float(np.sqrt(HD))

    nc = bacc.Bacc()

    # weights arrive host-pre-transposed to partition-major 3D layouts so
    # every DMA is 128 large contiguous descriptors (fast issue, line rate)
    xT = nc.declare_dram_parameter("xT", [C_, T_], bf16, isOutput=False)
    wq = nc.declare_dram_parameter("wq", [P, KT, GH * HD], bf16, isOutput=False)
    wk = nc.declare_dram_parameter("wk", [P, KT, HD], bf16, isOutput=False)
    wv = nc.declare_dram_parameter("wv", [P, KT, HD], bf16, isOutput=False)
    wg = nc.declare_dram_parameter("wg", [GATE_CH, 1], bf16, isOutput=False)
    ve2 = nc.declare_dram_parameter("ve2", [P, T_ // P, HD], bf16, isOutput=False)
    wo = nc.declare_dram_parameter("wo", [P, GH, C_], bf16, isOutput=False)
    ccd = nc.declare_dram_parameter("cc", [P, T_], bf16, isOutput=False)
    ssd = nc.declare_dram_parameter("ss", [P, T_], bf16, isOutput=False)
    tlo = nc.declare_dram_parameter("tlo", [P, P], bf16, isOutput=False)
    thi = nc.declare_dram_parameter("thi", [P, P], bf16, isOutput=False)
    idr = nc.declare_dram_parameter("identr", [P, GH * P], bf16, isOutput=False)
    idf = nc.declare_dram_parameter("identf", [P, P], f32, isOutput=False)
    out_d = nc.declare_dram_parameter("out", [T_, C_], bf16, isOutput=True)

    with tile.TileContext(nc) as tc:
        with (
            tc.tile_pool(name="singles", bufs=1) as sg,
            tc.tile_pool(name="work", bufs=2) as wk_pool,
            tc.tile_pool(name="work3", bufs=4) as w3_pool,
            tc.tile_pool(name="attw", bufs=5) as aw,
            tc.tile_pool(name="acc", bufs=2) as acc_pool,
            tc.tile_pool(name="yup", bufs=3) as yu_pool,
            tc.tile_pool(name="outp", bufs=3) as op_pool,
            tc.tile_pool(name="psum", bufs=8, space="PSUM") as pp,
        ):
            # ---- persistent inputs -------------------------------------
            # A dma_start occupies its issuing sequencer ~650ns+ and only ~8
            # DMAs can be in flight, so: the two HWDGE rings (sync/scalar)
            # carry the wave-0 critical path (wk + xt tiles, alternating, in
            # consumption order); everything non-critical queues behind or
            # goes to gpsimd's SWDGE ring.
            wq_sb = sg.tile([P, KT, GH * HD], bf16, tag="wq")
            wk_sb = sg.tile([P, KT, HD], bf16, tag="wk")
            wv_sb = sg.tile([P, KT, HD], bf16, tag="wv")
            nc.scalar.dma_start(out=wk_sb[:], in_=wk[:])
            nc.scalar.dma_start(out=wv_sb[:], in_=wv[:])
            xt = []
            xt_eng = [nc.sync, nc.scalar]
            for kt in range(KT):
                t_ = sg.tile([P, T_], bf16, tag=f"xt{kt}")
                xt_eng[kt % 2].dma_start(out=t_[:], in_=xT[kt * P:(kt + 1) * P, :])
                xt.append(t_)
            wg_sb = sg.tile([GATE_CH, 1], bf16, tag="wg")
            nc.gpsimd.dma_start(out=wg_sb[:], in_=wg[:])
            # wq in kt-quarters so wave 2's q-head matmuls can chase arrival
            for qtr in range(4):
                nc.scalar.dma_start(out=wq_sb[:, qtr * 4:(qtr + 1) * 4, :],
                                    in_=wq[:, qtr * 4:(qtr + 1) * 4, :])
            wo_sb = sg.tile([P, GH, C_], bf16, tag="wo")
            nc.sync.dma_start(out=wo_sb[:, 0:2, :], in_=wo[:, 0:2, :])
            nc.sync.dma_start(out=wo_sb[:, 2:4, :], in_=wo[:, 2:4, :])
            # constants not needed until the first tails — issued after
            # wave 0's matmuls so they don't compete with the xT stream
            ve2_sb = sg.tile([P, NT, HD], bf16, tag="ve2")
            cc_sb = sg.tile([P, T_], bf16, tag="cc")
            ss_sb = sg.tile([P, T_], bf16, tag="ss")
            tlo_sb = sg.tile([P, P], bf16, tag="tlo")
            thi_sb = sg.tile([P, P], bf16, tag="thi")
            idr_sb = sg.tile([P, GH * P], bf16, tag="idr")
            idf_sb = sg.tile([P, P], f32, tag="idf")

            def emit_const_dmas():
                nc.gpsimd.dma_start(out=ve2_sb[:], in_=ve2[:])
                nc.gpsimd.dma_start(out=cc_sb[:], in_=ccd[:])
                nc.gpsimd.dma_start(out=ss_sb[:], in_=ssd[:])
                nc.gpsimd.dma_start(out=idf_sb[:], in_=idf[:])
                nc.gpsimd.dma_start(out=tlo_sb[:], in_=tlo[:])
                nc.gpsimd.dma_start(out=thi_sb[:], in_=thi[:])
                nc.gpsimd.dma_start(out=idr_sb[:], in_=idr[:])

            ones128 = sg.tile([P, P], bf16, tag="ones128")
            nc.vector.memset(ones128[:], 1.0)
            eps_sb = sg.tile([P, 1], f32, tag="epsb")
            nc.vector.memset(eps_sb[:], 1e-6)
            # for k-head rms: fold the 1/sqrt(d) score scale into the rms
            # denominator exactly: ISQ/sqrt(ssq/HD+eps) = 1/sqrt(ssq+eps*HD)
            epsk_sb = sg.tile([P, 1], f32, tag="epskb")
            nc.vector.memset(epsk_sb[:], 1e-6 * HD)

            # persistent intermediates
            qhat = sg.tile([P, GH, T_], bf16, tag="qhat")   # normalized roped q, [d, h, t]
            khat = sg.tile([P, T_], bf16, tag="khat")       # normalized roped k * isq
            vsb = sg.tile([P, NT, HD], bf16, tag="vsb")     # gated v, [tok, tt, d]

            TS = T_ // 512  # 512-wide token slices

            # ---- projections + rope + rmsnorm for k/q heads and vT -----
            # Emitted as kt-major WAVES of 3 output groups: the PE chases the
            # xT DMAs tile-by-tile during the ramp, and each wave's dependent
            # tail work (rope/rms/broadcast) is batched behind the next
            # wave's matmuls so the PE stream never waits on DVE/ACT chains.
            # Wave order is ts-major with the k/v heads FIRST so that the
            # attention loop (emitted after all waves) never waits on tails:
            # attn(qi) needs k/v tiles up to qi+WT but q only for its own ts.
            def wave_mms(wave):
                items = []
                for (head, ts_) in wave:
                    sl = slice(ts_ * 512, ts_ * 512 + 512)
                    ps = pp.tile([P, 512], f32, tag="pb",
                                 name=f"ps{head}_{ts_}")
                    items.append((head, sl, ps))
                for kt in range(KT):
                    for gi, (head, ts_) in enumerate(wave):
                        if head == 0:
                            w_ap = wk_sb[:, kt, :]
                        elif head == GH + 1:
                            w_ap = wv_sb[:, kt, :]
                        else:
                            w_ap = wq_sb[:, kt, (head - 1) * HD:head * HD]
                        nc.tensor.matmul(
                            items[gi][2][:], lhsT=w_ap,
                            rhs=xt[kt][:, items[gi][1]],
                            start=(kt == 0), stop=(kt == KT - 1),
                        )
                return items

            # sigmoid gates for all token tiles, computed up front (the gate
            # matmuls fill the DMA-paced start of wave 0)
            gcols = sg.tile([P, NT], f32, tag="gcols")

            def emit_gates():
                for tt in range(NT):
                    tsl = slice(tt * P, (tt + 1) * P)
                    gps = pp.tile([P, 1], f32, tag="pb")
                    nc.tensor.matmul(gps[:], lhsT=xt[0][0:GATE_CH, tsl],
                                     rhs=wg_sb[:], start=True, stop=True)
                    nc.scalar.activation(gcols[:, tt:tt + 1], gps[:],
                                         AF.Sigmoid)

            def v_tail(head, sl, ps):
                # vT psum [d, tok] -> sbuf f32, PE-transpose each 128-tok
                # block to natural [tok, d], then add the sigmoid-gated ve.
                vt = wk_pool.tile([P, 512], f32, tag="vt")
                nc.vector.tensor_copy(vt[:], ps[:])
                for i in range(4):
                    tt = sl.start // P + i
                    tp = pp.tile([P, P], f32, tag="pb")
                    nc.tensor.transpose(tp[:], vt[:, i * P:(i + 1) * P],
                                        idf_sb[:])
                    # v = ve2 * sigmoid(g) + v_proj (ve2 pre-scaled by 2)
                    nc.vector.scalar_tensor_tensor(
                        out=vsb[:, tt, :], in0=ve2_sb[:, tt, :],
                        scalar=gcols[:, tt:tt + 1],
                        in1=tp[:], op0=ALU.mult, op1=ALU.add,
                    )

            def wave_tails(items, on_gp=False):
                # on_gp: route the rope arithmetic to the (otherwise idle)
                # GpSimd so late tails don't contend with the attention
                # phase's DVE work. GpSimd can't read PSUM, so it gets a
                # scalar-engine copy first (which also frees the bank early).
                ve = nc.gpsimd if on_gp else nc.vector
                t1 = []
                for (head, sl, ps) in items:
                    if head == GH + 1:
                        v_tail(head, sl, ps)
                        continue
                    # rope: qr = ps*cc + swap(ps)*ss  (ss carries the sign).
                    # on_gp: GpSimd can't read PSUM and needs equal base
                    # partitions, so the scalar engine makes a plain and a
                    # half-swapped SBUF copy first; everything else runs on
                    # GpSimd so the attention-phase DVE stays clear.
                    if on_gp:
                        psb = wk_pool.tile([P, 512], f32, tag="psb", bufs=1)
                        nc.scalar.copy(psb[:], ps[:])
                        psw = wk_pool.tile([P, 512], f32, tag="psw", bufs=1)
                        nc.scalar.copy(psw[0:64, :], ps[64:128, :])
                        nc.scalar.copy(psw[64:128, :], ps[0:64, :])
                    qr = w3_pool.tile([P, 512], f32, tag="qr")
                    ve.tensor_mul(qr[:], (psb if on_gp else ps)[:],
                                  cc_sb[:, sl])
                    qs = wk_pool.tile([P, 512], f32, tag="qs")
                    if on_gp:
                        ve.tensor_mul(qs[:], psw[:], ss_sb[:, sl])
                    else:
                        nc.vector.tensor_mul(qs[0:64, :], ps[64:128, :],
                                             ss_sb[0:64, sl])
                        nc.vector.tensor_mul(qs[64:128, :], ps[0:64, :],
                                             ss_sb[64:128, sl])
                    ve.tensor_add(qr[:], qr[:], qs[:])
                    q2 = wk_pool.tile([P, 512], bf16, tag="q2")
                    nc.gpsimd.tensor_mul(q2[:], qr[:], qr[:])
                    t1.append((head, sl, qr, q2))
                # sum-of-squares reduced AND broadcast in one bf16 matmul:
                # all-ones lhsT makes every psum row the partition-sum.
                ssqs = []
                for (head, sl, qr, q2) in t1:
                    ssqb = pp.tile([P, 512], f32, tag="pb")
                    nc.tensor.matmul(ssqb[:], lhsT=ones128[:], rhs=q2[:],
                                     start=True, stop=True)
                    ssqs.append(ssqb)
                rows = []
                for (head, sl, qr, q2), ssqb in zip(t1, ssqs):
                    srow = w3_pool.tile([P, 512], f32, tag="srow")
                    if head == 0:
                        # k: 1/sqrt(ssq + eps*HD) == ISQ/sqrt(msq + eps)
                        nc.scalar.activation(srow[:], ssqb[:], AF.Sqrt,
                                             bias=epsk_sb[:], scale=1.0)
                    else:
                        nc.scalar.activation(srow[:], ssqb[:], AF.Sqrt,
                                             bias=eps_sb[:], scale=1.0 / HD)
                    rows.append(srow)
                rrs = []
                for (head, sl, qr, q2), srow in zip(t1, rows):
                    rr = w3_pool.tile([P, 512], f32, tag="rr")
                    nc.vector.reciprocal_approx_fast(rr[:], srow[:])
                    rrs.append(rr)
                for (head, sl, qr, q2), rr in zip(t1, rrs):
                    dest = khat[:, sl] if head == 0 else qhat[:, head - 1, sl]
                    ve.tensor_mul(dest, qr[:], rr[:])

            K_, V_ = 0, GH + 1
            # wave 0 takes 6 k/v groups: it is paced by the arriving xT
            # tiles, so consuming 6 matmuls per kt matches the DMA rate
            waves = [
                [(K_, 0), (K_, 1), (K_, 2), (V_, 0), (V_, 1), (V_, 2)],
                [(K_, 3), (V_, 3), (1, 0)],
                [(2, 0), (3, 0), (4, 0)],
                [(1, 1), (2, 1), (3, 1)],
                [(4, 1), (1, 2), (2, 2)],
                [(3, 2), (4, 2), (1, 3)],
                [(2, 3), (3, 3), (4, 3)],
            ]

            CO = C_ // 512  # output column chunks
            # All 4 q-heads are fused into one 512-wide moving operand:
            # scores / exp / den / PV are each ONE N=512 instruction per
            # (qi, kt), so LDWEIGHTS fully hides under the matmul stream.
            yps = {}
            yus = {}
            denbs = {}
            rdbs = {}

            def attn_scores_k(qi, kk):
                ktc = min(WT + 1, NT - qi)
                qs4 = qhat[:, :, qi * P:(qi + 1) * P]   # [d, (h, q)] = 512 wide
                kt = qi + kk
                sp = pp.tile([P, GH * P], f32, tag="pb")
                masked = (kk == 0) or (kk == WT and ktc == WT + 1)
                nc.tensor.matmul(
                    sp[:], lhsT=khat[:, kt * P:(kt + 1) * P], rhs=qs4,
                    start=True, stop=not masked,
                )
                if masked:
                    # band-mask bias (-3e4 outside band): psum += bias.T @ I_rep
                    nc.tensor.matmul(
                        sp[:], lhsT=tlo_sb[:] if kk == 0 else thi_sb[:],
                        rhs=idr_sb[:], start=False, stop=True,
                    )
                pt = aw.tile([P, GH * P], bf16, tag="pT")
                nc.scalar.activation(pt[:], sp[:], AF.Exp)
                return pt

            def attn_pv_k(qi, kk, pt, pts):
                ktc = min(WT + 1, NT - qi)
                if kk == 0:
                    yps[qi] = pp.tile([P, GH * P], f32, tag="pb",
                                      name=f"yp{qi}")
                    denbs[qi] = pp.tile([P, GH * P], f32, tag="pb",
                                        name=f"denb{qi}")
                kt = qi + kk
                nc.tensor.matmul(
                    yps[qi][:], lhsT=vsb[:, kt, :], rhs=pt[:],
                    start=(kk == 0), stop=(kk == ktc - 1),
                )
                # denominator: reduce over k AND broadcast to all 128 rows by
                # accumulating all-ones matmuls alongside the PV stream (the
                # pt operand is already in-flight; no extra dependencies)
                nc.tensor.matmul(
                    denbs[qi][:], lhsT=ones128[:], rhs=pt[:],
                    start=(kk == 0), stop=(kk == ktc - 1),
                )
                if kk == ktc - 1:
                    # yut on the scalar engine: it runs in parallel with the
                    # DVE reciprocal (independent inputs), so yq starts sooner
                    yut = yu_pool.tile([P, GH * P], f32, tag="yu")
                    nc.scalar.copy(yut[:], yps[qi][:])
                    yus[qi] = yut
                    rdb = yu_pool.tile([P, GH * P], f32, tag="rdb", bufs=2)
                    nc.vector.reciprocal_approx_fast(rdb[:], denbs[qi][:])
                    yq = op_pool.tile([P, GH * P], bf16, tag="yq")
                    nc.vector.tensor_mul(yq[:], yut[:], rdb[:])
                    rdbs[qi] = yq

            def attn_out(qi, split_dma=False):
                qsl = slice(qi * P, (qi + 1) * P)
                yq = rdbs[qi]
                ob4 = op_pool.tile([P, CO, 512], bf16, tag="ob4", bufs=2)
                for co in range(CO):
                    osl = slice(co * 512, co * 512 + 512)
                    ops = pp.tile([P, 512], f32, tag="pb")
                    for h in range(GH):
                        nc.tensor.matmul(
                            ops[:], lhsT=yq[:, h * P:(h + 1) * P],
                            rhs=wo_sb[:, h, osl],
                            start=(h == 0), stop=(h == GH - 1),
                        )
                    nc.vector.tensor_copy(out=ob4[:, co, :], in_=ops[:])
                    if split_dma and co == 1:
                        nc.gpsimd.dma_start(out=out_d[qsl, 0:1024],
                                            in_=ob4[:, 0:2, :])
                if split_dma:
                    nc.gpsimd.dma_start(out=out_d[qsl, 1024:2048],
                                        in_=ob4[:, 2:4, :])
                else:
                    nc.gpsimd.dma_start(out=out_d[qsl, :], in_=ob4[:])

            PV_DEPTH = 3
            pv_queue = deque()
            pts = {}   # qi -> list of pt tiles (for the first-pair dedup add)
            done_out = set()

            def attn_emit(qi, drain=True):
                ktc = min(WT + 1, NT - qi)
                for kk in range(ktc):
                    pt = attn_scores_k(qi, kk)
                    pts.setdefault(qi, []).append(pt)
                    if len(pv_queue) >= PV_DEPTH:
                        attn_pv_k(*pv_queue.popleft(), pts)
                    pv_queue.append((qi, kk, pt))
                    # emit the out-projection for qi-1 once its normalized
                    # yq (den chain emitted at qi-1's drain) has lead time
                    if qi > 0 and kk >= min(3, ktc - 1) and (qi - 1) in rdbs \
                            and (qi - 1) not in done_out:
                        done_out.add(qi - 1)
                        attn_out(qi - 1)
                # drain at every qi boundary: the den/reciprocal chain is
                # emitted ~3 slots earlier, so the next qi's scores cover its
                # latency and attn_out(qi) never stalls the PE queue
                while pv_queue:
                    attn_pv_k(*pv_queue.popleft(), pts)

            # Schedule: tails(w) ride one wave behind; the first attention
            # q-tiles slot between the last waves so the final tails' DVE
            # chains overlap attention matmuls instead of stalling the PE.
            # Tails marked "g" run their rope math on GpSimd to keep the
            # attention phase's DVE queue clear.
            sched = [("w", 0), ("w", 1), ("t", 0), ("w", 2), ("t", 1),
                     ("w", 3), ("t", 2), ("w", 4), ("t", 3), ("w", 5),
                     ("t", 4), ("a", 0), ("w", 6), ("g", 5), ("a", 1),
                     ("a", 2), ("g", 6), ("a", 3)] + \
                    [("a", qi) for qi in range(4, NT)]
            wave_items = {}
            for kind, idx in sched:
                if kind == "w":
                    wave_items[idx] = wave_mms(waves[idx])
                    if idx == 0:
                        emit_gates()
                        emit_const_dmas()
                elif kind in ("t", "g"):
                    wave_tails(wave_items.pop(idx), on_gp=(kind == "g"))
                else:
                    attn_emit(idx)
            for qi in range(NT):
                if qi not in done_out:
                    attn_out(qi, split_dma=True)

    return nc


def _get_program(T_=T, C_=C, win=WINDOW):
    key = (T_, C_, win)
    if key not in _PROGRAM_CACHE:
        nc = build_program(T_, C_, win)
        nc.finalize()
        _PROGRAM_CACHE[key] = nc
    return _PROGRAM_CACHE[key]


def make_in_maps(x, ve, cos, sin, Wq, Wk, Wv, Wg, Wo):
    """Build the 8 per-core input dicts (host-side sharding/layout prep)."""
    cosT = np.ascontiguousarray(cos[:, 0, :].T).astype(np.float32)  # [64, T]
    sinT = np.ascontiguousarray(sin[:, 0, :].T).astype(np.float32)
    cc = np.concatenate([cosT, cosT], axis=0)            # [128, T]
    ss = np.concatenate([sinT, -sinT], axis=0)           # [128, T]
    # additive mask biases for the S^T diagonal/far tiles, pre-transposed
    # (they enter the scores as lhsT with an identity rhs: psum += bias.T)
    neg = np.float32(-30000.0)
    bias_lo = np.where(np.arange(P)[:, None] >= np.arange(P)[None, :], 0.0, neg)
    bias_hi = np.where(np.arange(P)[:, None] < np.arange(P)[None, :], 0.0, neg)
    tlo = np.ascontiguousarray(bias_lo.T).astype(BF16)
    thi = np.ascontiguousarray(bias_hi.T).astype(BF16)
    identr = np.tile(np.eye(P, dtype=np.float32), (1, GH)).astype(BF16)

    def pmajor(w):
        # [(o p), n] -> [p, o, n] contiguous (partition-major for the DMA)
        o = w.shape[0] // P
        return np.ascontiguousarray(
            w.reshape(o, P, w.shape[1]).transpose(1, 0, 2))

    in_maps = []
    for core in range(N_CORES):
        b, g = divmod(core, N_KV)
        in_maps.append({
            "xT": np.ascontiguousarray(x[b].T).astype(BF16),
            "wq": pmajor(Wq[:, g * GH * HD:(g + 1) * GH * HD].astype(BF16)),
            "wk": pmajor(Wk[:, g * HD:(g + 1) * HD].astype(BF16)),
            "wv": pmajor(Wv[:, g * HD:(g + 1) * HD].astype(BF16)),
            "wg": np.ascontiguousarray(Wg[:, g:g + 1]).astype(BF16),
            "ve2": pmajor((2.0 * ve[b][:, g * HD:(g + 1) * HD]).astype(BF16)),
            "wo": pmajor(Wo[g * GH * HD:(g + 1) * GH * HD, :].astype(BF16)),
            "cc": cc.astype(BF16), "ss": ss.astype(BF16),
            "tlo": tlo, "thi": thi, "identr": identr,
            "identf": np.eye(P, dtype=np.float32),
        })
    return in_maps


def kernel(x, ve, cos, sin, Wq, Wk, Wv, Wg, Wo, window):
    assert int(window) == WINDOW
    x, ve, cos, sin, Wq, Wk, Wv, Wg, Wo = (
        np.asarray(a, dtype=np.float32)
        for a in (x, ve, cos, sin, Wq, Wk, Wv, Wg, Wo))
    assert x.shape == (B, T, C)
    from concourse.bass_utils import run_bass_kernel_spmd

    nc = _get_program()
    in_maps = make_in_maps(x, ve, cos, sin, Wq, Wk, Wv, Wg, Wo)
    res = run_bass_kernel_spmd(nc, in_maps, core_ids=list(range(N_CORES)))
    out = np.zeros((B, T, C), dtype=np.float32)
    for core in range(N_CORES):
        b = core // N_KV
        out[b] += np.asarray(res.results[core]["out"], dtype=np.float32)
    return out


# revision 50
# speedup vs baseline: 1.0111x; 1.0111x over previous
"""Trainium2 Bass kernel for a GQA sliding-window attention layer.

Reference computation (B=2, T=2048, C=2048, 16 Q heads / 4 KV heads, d=128):
    q = x @ Wq; k = x @ Wk; v = x @ Wv (+ sigmoid-gated value embedding)
    q, k = rmsnorm(rope(q)), rmsnorm(rope(k))
    scores masked to the band 0 <= j - i < window (=1024), softmax over j
    out = (p @ v) @ Wo

Sharding: 8 cores = 2 batches x 4 KV groups.  Each core computes its 4 Q
heads / 1 KV head for one batch and a partial output (its 512-row slice of
the Wo contraction); the host sums the 4 partials per batch.

Layout strategy per core:
  - xT (C x T, bf16) resident in SBUF; all projections contract over C.
  - q̂T / k̂T kept [d=128 partitions, T free]; scores computed transposed
    (S^T tiles [kj, qi]) so that P^T feeds the PV matmul directly with v in
    natural [token, d] layout (no P transposes).
  - softmax has no max-subtraction: rms-normalized q,k bound |score| by
    sqrt(128), so exp is safe in fp32.
  - partition-broadcast reductions (rms rows, softmax denominators) are done
    in ONE cheap bf16 matmul with an all-ones [128,128] lhsT: every output
    row equals the partition-sum, so no separate row-extract + re-broadcast.
  - softmax denominators are pre-summed over k-tiles on the DVE (bf16) so
    the PE does a single reduction matmul per q-tile instead of nine.
"""

import numpy as np
import ml_dtypes
from collections import deque

BF16 = ml_dtypes.bfloat16

# Problem dims (hardcoded per contest rules)
B, T, C = 2, 2048, 2048
N_HEAD, N_KV, HD, GATE_CH = 16, 4, 128, 32
WINDOW = 1024
P = 128
GH = N_HEAD // N_KV  # q heads per kv head (= per core)
N_CORES = 8

_PROGRAM_CACHE = {}


def build_program(T_=T, C_=C, win=WINDOW):
    import concourse.mybir as mybir
    import concourse.tile as tile
    from concourse import bacc

    dt = mybir.dt
    f32 = dt.float32
    bf16 = dt.bfloat16
    AF = mybir.ActivationFunctionType
    ALU = mybir.AluOpType

    NT = T_ // P          # token tiles
    KT = C_ // P          # contraction tiles
    WT = win // P         # window tiles
    ISQ = 1.0 / float(np.sqrt(HD))

    nc = bacc.Bacc()

    # weights arrive host-pre-transposed to partition-major 3D layouts so
    # every DMA is 128 large contiguous descriptors (fast issue, line rate)
    xT = nc.declare_dram_parameter("xT", [C_, T_], bf16, isOutput=False)
    wq = nc.declare_dram_parameter("wq", [P, KT, GH * HD], bf16, isOutput=False)
    wk = nc.declare_dram_parameter("wk", [P, KT, HD], bf16, isOutput=False)
    wv = nc.declare_dram_parameter("wv", [P, KT, HD], bf16, isOutput=False)
    wg = nc.declare_dram_parameter("wg", [GATE_CH, 1], bf16, isOutput=False)
    ve2 = nc.declare_dram_parameter("ve2", [P, T_ // P, HD], bf16, isOutput=False)
    wo = nc.declare_dram_parameter("wo", [P, GH, C_], bf16, isOutput=False)
    ccd = nc.declare_dram_parameter("cc", [P, T_], bf16, isOutput=False)
    ssd = nc.declare_dram_parameter("ss", [P, T_], bf16, isOutput=False)
    tlo = nc.declare_dram_parameter("tlo", [P, P], bf16, isOutput=False)
    thi = nc.declare_dram_parameter("thi", [P, P], bf16, isOutput=False)
    idr = nc.declare_dram_parameter("identr", [P, GH * P], bf16, isOutput=False)
    idf = nc.declare_dram_parameter("identf", [P, P], f32, isOutput=False)
    out_d = nc.declare_dram_parameter("out", [T_, C_], bf16, isOutput=True)

    with tile.TileContext(nc) as tc:
        with (
            tc.tile_pool(name="singles", bufs=1) as sg,
            tc.tile_pool(name="work", bufs=2) as wk_pool,
            tc.tile_pool(name="work3", bufs=4) as w3_pool,
            tc.tile_pool(name="attw", bufs=5) as aw,
            tc.tile_pool(name="acc", bufs=2) as acc_pool,
            tc.tile_pool(name="yup", bufs=3) as yu_pool,
            tc.tile_pool(name="outp", bufs=3) as op_pool,
            tc.tile_pool(name="psum", bufs=8, space="PSUM") as pp,
        ):
            # ---- persistent inputs -------------------------------------
            # A dma_start occupies its issuing sequencer ~650ns+ and only ~8
            # DMAs can be in flight, so: the two HWDGE rings (sync/scalar)
            # carry the wave-0 critical path (wk + xt tiles, alternating, in
            # consumption order); everything non-critical queues behind or
            # goes to gpsimd's SWDGE ring.
            wq_sb = sg.tile([P, KT, GH * HD], bf16, tag="wq")
            wk_sb = sg.tile([P, KT, HD], bf16, tag="wk")
            wv_sb = sg.tile([P, KT, HD], bf16, tag="wv")
            nc.scalar.dma_start(out=wk_sb[:], in_=wk[:])
            nc.scalar.dma_start(out=wv_sb[:], in_=wv[:])
            xt = []
            xt_eng = [nc.sync, nc.scalar]
            for kt in range(KT):
                t_ = sg.tile([P, T_], bf16, tag=f"xt{kt}")
                xt_eng[kt % 2].dma_start(out=t_[:], in_=xT[kt * P:(kt + 1) * P, :])
                xt.append(t_)
            wg_sb = sg.tile([GATE_CH, 1], bf16, tag="wg")
            nc.gpsimd.dma_start(out=wg_sb[:], in_=wg[:])
            # wq in kt-quarters so wave 2's q-head matmuls can chase arrival
            for qtr in range(4):
                nc.scalar.dma_start(out=wq_sb[:, qtr * 4:(qtr + 1) * 4, :],
                                    in_=wq[:, qtr * 4:(qtr + 1) * 4, :])
            wo_sb = sg.tile([P, GH, C_], bf16, tag="wo")
            nc.sync.dma_start(out=wo_sb[:, 0:2, :], in_=wo[:, 0:2, :])
            nc.sync.dma_start(out=wo_sb[:, 2:4, :], in_=wo[:, 2:4, :])
            # constants not needed until the first tails — issued after
            # wave 0's matmuls so they don't compete with the xT stream
            ve2_sb = sg.tile([P, NT, HD], bf16, tag="ve2")
            cc_sb = sg.tile([P, T_], bf16, tag="cc")
            ss_sb = sg.tile([P, T_], bf16, tag="ss")
            tlo_sb = sg.tile([P, P], bf16, tag="tlo")
            thi_sb = sg.tile([P, P], bf16, tag="thi")
            idr_sb = sg.tile([P, GH * P], bf16, tag="idr")
            idf_sb = sg.tile([P, P], f32, tag="idf")

            def emit_const_dmas():
                nc.gpsimd.dma_start(out=ve2_sb[:], in_=ve2[:])
                nc.gpsimd.dma_start(out=cc_sb[:], in_=ccd[:])
                nc.gpsimd.dma_start(out=ss_sb[:], in_=ssd[:])
                nc.gpsimd.dma_start(out=idf_sb[:], in_=idf[:])
                nc.gpsimd.dma_start(out=tlo_sb[:], in_=tlo[:])
                nc.gpsimd.dma_start(out=thi_sb[:], in_=thi[:])
                nc.gpsimd.dma_start(out=idr_sb[:], in_=idr[:])

            ones128 = sg.tile([P, P], bf16, tag="ones128")
            nc.vector.memset(ones128[:], 1.0)
            eps_sb = sg.tile([P, 1], f32, tag="epsb")
            nc.vector.memset(eps_sb[:], 1e-6)
            # for k-head rms: fold the 1/sqrt(d) score scale into the rms
            # denominator exactly: ISQ/sqrt(ssq/HD+eps) = 1/sqrt(ssq+eps*HD)
            epsk_sb = sg.tile([P, 1], f32, tag="epskb")
            nc.vector.memset(epsk_sb[:], 1e-6 * HD)

            # persistent intermediates
            qhat = sg.tile([P, GH, T_], bf16, tag="qhat")   # normalized roped q, [d, h, t]
            khat = sg.tile([P, T_], bf16, tag="khat")       # normalized roped k * isq
            vsb = sg.tile([P, NT, HD], bf16, tag="vsb")     # gated v, [tok, tt, d]

            TS = T_ // 512  # 512-wide token slices

            # ---- projections + rope + rmsnorm for k/q heads and vT -----
            # Emitted as kt-major WAVES of 3 output groups: the PE chases the
            # xT DMAs tile-by-tile during the ramp, and each wave's dependent
            # tail work (rope/rms/broadcast) is batched behind the next
            # wave's matmuls so the PE stream never waits on DVE/ACT chains.
            # Wave order is ts-major with the k/v heads FIRST so that the
            # attention loop (emitted after all waves) never waits on tails:
            # attn(qi) needs k/v tiles up to qi+WT but q only for its own ts.
            def wave_mms(wave):
                items = []
                for (head, ts_) in wave:
                    sl = slice(ts_ * 512, ts_ * 512 + 512)
                    ps = pp.tile([P, 512], f32, tag="pb",
                                 name=f"ps{head}_{ts_}")
                    items.append((head, sl, ps))
                for kt in range(KT):
                    for gi, (head, ts_) in enumerate(wave):
                        if head == 0:
                            w_ap = wk_sb[:, kt, :]
                        elif head == GH + 1:
                            w_ap = wv_sb[:, kt, :]
                        else:
                            w_ap = wq_sb[:, kt, (head - 1) * HD:head * HD]
                        nc.tensor.matmul(
                            items[gi][2][:], lhsT=w_ap,
                            rhs=xt[kt][:, items[gi][1]],
                            start=(kt == 0), stop=(kt == KT - 1),
                        )
                return items

            # sigmoid gates for all token tiles, computed up front (the gate
            # matmuls fill the DMA-paced start of wave 0)
            gcols = sg.tile([P, NT], f32, tag="gcols")

            def emit_gates():
                for tt in range(NT):
                    tsl = slice(tt * P, (tt + 1) * P)
                    gps = pp.tile([P, 1], f32, tag="pb")
                    nc.tensor.matmul(gps[:], lhsT=xt[0][0:GATE_CH, tsl],
                                     rhs=wg_sb[:], start=True, stop=True)
                    nc.scalar.activation(gcols[:, tt:tt + 1], gps[:],
                                         AF.Sigmoid)

            def v_tail(head, sl, ps):
                # vT psum [d, tok] -> sbuf f32, PE-transpose each 128-tok
                # block to natural [tok, d], then add the sigmoid-gated ve.
                vt = wk_pool.tile([P, 512], f32, tag="vt")
                nc.vector.tensor_copy(vt[:], ps[:])
                for i in range(4):
                    tt = sl.start // P + i
                    tp = pp.tile([P, P], f32, tag="pb")
                    nc.tensor.transpose(tp[:], vt[:, i * P:(i + 1) * P],
                                        idf_sb[:])
                    # v = ve2 * sigmoid(g) + v_proj (ve2 pre-scaled by 2)
                    nc.vector.scalar_tensor_tensor(
                        out=vsb[:, tt, :], in0=ve2_sb[:, tt, :],
                        scalar=gcols[:, tt:tt + 1],
                        in1=tp[:], op0=ALU.mult, op1=ALU.add,
                    )

            def wave_tails(items, on_gp=False):
                # on_gp: route the rope arithmetic to the (otherwise idle)
                # GpSimd so late tails don't contend with the attention
                # phase's DVE work. GpSimd can't read PSUM, so it gets a
                # scalar-engine copy first (which also frees the bank early).
                ve = nc.gpsimd if on_gp else nc.vector
                t1 = []
                for (head, sl, ps) in items:
                    if head == GH + 1:
                        v_tail(head, sl, ps)
                        continue
                    # rope: qr = ps*cc + swap(ps)*ss  (ss carries the sign).
                    # on_gp: GpSimd can't read PSUM and needs equal base
                    # partitions, so the scalar engine makes a plain and a
                    # half-swapped SBUF copy first; everything else runs on
                    # GpSimd so the attention-phase DVE stays clear.
                    if on_gp:
                        psb = wk_pool.tile([P, 512], f32, tag="psb", bufs=1)
                        nc.scalar.copy(psb[:], ps[:])
                        psw = wk_pool.tile([P, 512], f32, tag="psw", bufs=1)
                        nc.scalar.copy(psw[0:64, :], ps[64:128, :])
                        nc.scalar.copy(psw[64:128, :], ps[0:64, :])
                    qr = w3_pool.tile([P, 512], f32, tag="qr")
                    ve.tensor_mul(qr[:], (psb if on_gp else ps)[:],
                                  cc_sb[:, sl])
                    qs = wk_pool.tile([P, 512], f32, tag="qs")
                    if on_gp:
                        ve.tensor_mul(qs[:], psw[:], ss_sb[:, sl])
                    else:
                        nc.vector.tensor_mul(qs[0:64, :], ps[64:128, :],
                                             ss_sb[0:64, sl])
                        nc.vector.tensor_mul(qs[64:128, :], ps[0:64, :],
                                             ss_sb[64:128, sl])
                    ve.tensor_add(qr[:], qr[:], qs[:])
                    q2 = wk_pool.tile([P, 512], bf16, tag="q2")
                    nc.gpsimd.tensor_mul(q2[:], qr[:], qr[:])
                    t1.append((head, sl, qr, q2))
                # sum-of-squares reduced AND broadcast in one bf16 matmul:
                # all-ones lhsT makes every psum row the partition-sum.
                ssqs = []
                for (head, sl, qr, q2) in t1:
                    ssqb = pp.tile([P, 512], f32, tag="pb")
                    nc.tensor.matmul(ssqb[:], lhsT=ones128[:], rhs=q2[:],
                                     start=True, stop=True)
                    ssqs.append(ssqb)
                rows = []
                for (head, sl, qr, q2), ssqb in zip(t1, ssqs):
                    srow = w3_pool.tile([P, 512], f32, tag="srow")
                    if head == 0:
                        # k: 1/sqrt(ssq + eps*HD) == ISQ/sqrt(msq + eps)
                        nc.scalar.activation(srow[:], ssqb[:], AF.Sqrt,
                                             bias=epsk_sb[:], scale=1.0)
                    else:
                        nc.scalar.activation(srow[:], ssqb[:], AF.Sqrt,
                                             bias=eps_sb[:], scale=1.0 / HD)
                    rows.append(srow)
                rrs = []
                for (head, sl, qr, q2), srow in zip(t1, rows):
                    rr = w3_pool.tile([P, 512], f32, tag="rr")
                    nc.vector.reciprocal_approx_fast(rr[:], srow[:])
                    rrs.append(rr)
                for (head, sl, qr, q2), rr in zip(t1, rrs):
                    dest = khat[:, sl] if head == 0 else qhat[:, head - 1, sl]
                    ve.tensor_mul(dest, qr[:], rr[:])

            K_, V_ = 0, GH + 1
            # wave 0 takes 6 k/v groups: it is paced by the arriving xT
            # tiles, so consuming 6 matmuls per kt matches the DMA rate
            waves = [
                [(K_, 0), (K_, 1), (K_, 2), (V_, 0), (V_, 1), (V_, 2)],
                [(K_, 3), (V_, 3), (1, 0)],
                [(2, 0), (3, 0), (4, 0)],
                [(1, 1), (2, 1), (3, 1)],
                [(4, 1), (1, 2), (2, 2)],
                [(3, 2), (4, 2), (1, 3)],
                [(2, 3), (3, 3), (4, 3)],
            ]

            CO = C_ // 512  # output column chunks
            # All 4 q-heads are fused into one 512-wide moving operand:
            # scores / exp / den / PV are each ONE N=512 instruction per
            # (qi, kt), so LDWEIGHTS fully hides under the matmul stream.
            yps = {}
            yus = {}
            denbs = {}
            rdbs = {}

            def attn_scores_k(qi, kk):
                ktc = min(WT + 1, NT - qi)
                qs4 = qhat[:, :, qi * P:(qi + 1) * P]   # [d, (h, q)] = 512 wide
                kt = qi + kk
                sp = pp.tile([P, GH * P], f32, tag="pb")
                masked = (kk == 0) or (kk == WT and ktc == WT + 1)
                nc.tensor.matmul(
                    sp[:], lhsT=khat[:, kt * P:(kt + 1) * P], rhs=qs4,
                    start=True, stop=not masked,
                )
                if masked:
                    # band-mask bias (-3e4 outside band): psum += bias.T @ I_rep
                    nc.tensor.matmul(
                        sp[:], lhsT=tlo_sb[:] if kk == 0 else thi_sb[:],
                        rhs=idr_sb[:], start=False, stop=True,
                    )
                pt = aw.tile([P, GH * P], bf16, tag="pT")
                nc.scalar.activation(pt[:], sp[:], AF.Exp)
                return pt

            def attn_pv_k(qi, kk, pt, pts):
                ktc = min(WT + 1, NT - qi)
                if kk == 0:
                    yps[qi] = pp.tile([P, GH * P], f32, tag="pb",
                                      name=f"yp{qi}")
                    denbs[qi] = pp.tile([P, GH * P], f32, tag="pb",
                                        name=f"denb{qi}")
                kt = qi + kk
                nc.tensor.matmul(
                    yps[qi][:], lhsT=vsb[:, kt, :], rhs=pt[:],
                    start=(kk == 0), stop=(kk == ktc - 1),
                )
                # denominator: reduce over k AND broadcast to all 128 rows by
                # accumulating all-ones matmuls alongside the PV stream (the
                # pt operand is already in-flight; no extra dependencies)
                nc.tensor.matmul(
                    denbs[qi][:], lhsT=ones128[:], rhs=pt[:],
                    start=(kk == 0), stop=(kk == ktc - 1),
                )
                if kk == ktc - 1:
                    yut = yu_pool.tile([P, GH * P], f32, tag="yu")
                    nc.vector.tensor_copy(yut[:], yps[qi][:])
                    yus[qi] = yut
                    rdb = yu_pool.tile([P, GH * P], f32, tag="rdb", bufs=2)
                    nc.vector.reciprocal_approx_fast(rdb[:], denbs[qi][:])
                    yq = op_pool.tile([P, GH * P], bf16, tag="yq")
                    nc.vector.tensor_mul(yq[:], yut[:], rdb[:])
                    rdbs[qi] = yq

            def attn_out(qi):
                qsl = slice(qi * P, (qi + 1) * P)
                yq = rdbs[qi]
                ob4 = op_pool.tile([P, CO, 512], bf16, tag="ob4", bufs=2)
                for co in range(CO):
                    osl = slice(co * 512, co * 512 + 512)
                    ops = pp.tile([P, 512], f32, tag="pb")
                    for h in range(GH):
                        nc.tensor.matmul(
                            ops[:], lhsT=yq[:, h * P:(h + 1) * P],
                            rhs=wo_sb[:, h, osl],
                            start=(h == 0), stop=(h == GH - 1),
                        )
                    if co % 2 == 0:
                        nc.vector.tensor_copy(out=ob4[:, co, :], in_=ops[:])
                    else:
                        nc.scalar.copy(out=ob4[:, co, :], in_=ops[:])
                nc.gpsimd.dma_start(out=out_d[qsl, :], in_=ob4[:])

            PV_DEPTH = 3
            pv_queue = deque()
            pts = {}   # qi -> list of pt tiles (for the first-pair dedup add)
            done_out = set()

            def attn_emit(qi, drain=True):
                ktc = min(WT + 1, NT - qi)
                for kk in range(ktc):
                    pt = attn_scores_k(qi, kk)
                    pts.setdefault(qi, []).append(pt)
                    if len(pv_queue) >= PV_DEPTH:
                        attn_pv_k(*pv_queue.popleft(), pts)
                    pv_queue.append((qi, kk, pt))
                    # emit the out-projection for qi-1 once its normalized
                    # yq (den chain emitted at qi-1's drain) has lead time
                    if qi > 0 and kk >= min(3, ktc - 1) and (qi - 1) in rdbs \
                            and (qi - 1) not in done_out:
                        done_out.add(qi - 1)
                        attn_out(qi - 1)
                # drain at every qi boundary: the den/reciprocal chain is
                # emitted ~3 slots earlier, so the next qi's scores cover its
                # latency and attn_out(qi) never stalls the PE queue
                while pv_queue:
                    attn_pv_k(*pv_queue.popleft(), pts)

            # Schedule: tails(w) ride one wave behind; the first attention
            # q-tiles slot between the last waves so the final tails' DVE
            # chains overlap attention matmuls instead of stalling the PE.
            # Tails marked "g" run their rope math on GpSimd to keep the
            # attention phase's DVE queue clear.
            sched = [("w", 0), ("w", 1), ("t", 0), ("w", 2), ("t", 1),
                     ("w", 3), ("t", 2), ("w", 4), ("t", 3), ("w", 5),
                     ("t", 4), ("a", 0), ("w", 6), ("g", 5), ("a", 1),
                     ("a", 2), ("g", 6), ("a", 3)] + \
                    [("a", qi) for qi in range(4, NT)]
            wave_items = {}
            for kind, idx in sched:
                if kind == "w":
                    wave_items[idx] = wave_mms(waves[idx])
                    if idx == 0:
                        emit_gates()
                        emit_const_dmas()
                elif kind in ("t", "g"):
                    wave_tails(wave_items.pop(idx), on_gp=(kind == "g"))
                else:
                    attn_emit(idx)
            for qi in range(NT):
                if qi not in done_out:
                    attn_out(qi)

    return nc


def _get_program(T_=T, C_=C, win=WINDOW):
    key = (T_, C_, win)
    if key not in _PROGRAM_CACHE:
        nc = build_program(T_, C_, win)
        nc.finalize()
        _PROGRAM_CACHE[key] = nc
    return _PROGRAM_CACHE[key]


def make_in_maps(x, ve, cos, sin, Wq, Wk, Wv, Wg, Wo):
    """Build the 8 per-core input dicts (host-side sharding/layout prep)."""
    cosT = np.ascontiguousarray(cos[:, 0, :].T).astype(np.float32)  # [64, T]
    sinT = np.ascontiguousarray(sin[:, 0, :].T).astype(np.float32)
    cc = np.concatenate([cosT, cosT], axis=0)            # [128, T]
    ss = np.concatenate([sinT, -sinT], axis=0)           # [128, T]
    # additive mask biases for the S^T diagonal/far tiles, pre-transposed
    # (they enter the scores as lhsT with an identity rhs: psum += bias.T)
    neg = np.float32(-30000.0)
    bias_lo = np.where(np.arange(P)[:, None] >= np.arange(P)[None, :], 0.0, neg)
    bias_hi = np.where(np.arange(P)[:, None] < np.arange(P)[None, :], 0.0, neg)
    tlo = np.ascontiguousarray(bias_lo.T).astype(BF16)
    thi = np.ascontiguousarray(bias_hi.T).astype(BF16)
    identr = np.tile(np.eye(P, dtype=np.float32), (1, GH)).astype(BF16)

    def pmajor(w):
        # [(o p), n] -> [p, o, n] contiguous (partition-major for the DMA)
        o = w.shape[0] // P
        return np.ascontiguousarray(
            w.reshape(o, P, w.shape[1]).transpose(1, 0, 2))

    in_maps = []
    for core in range(N_CORES):
        b, g = divmod(core, N_KV)
        in_maps.append({
            "xT": np.ascontiguousarray(x[b].T).astype(BF16),
            "wq": pmajor(Wq[:, g * GH * HD:(g + 1) * GH * HD].astype(BF16)),
            "wk": pmajor(Wk[:, g * HD:(g + 1) * HD].astype(BF16)),
            "wv": pmajor(Wv[:, g * HD:(g + 1) * HD].astype(BF16)),
            "wg": np.ascontiguousarray(Wg[:, g:g + 1]).astype(BF16),
            "ve2": pmajor((2.0 * ve[b][:, g * HD:(g + 1) * HD]).astype(BF16)),
            "wo": pmajor(Wo[g * GH * HD:(g + 1) * GH * HD, :].astype(BF16)),
            "cc": cc.astype(BF16), "ss": ss.astype(BF16),
            "tlo": tlo, "thi": thi, "identr": identr,
            "identf": np.eye(P, dtype=np.float32),
        })
    return in_maps


def kernel(x, ve, cos, sin, Wq, Wk, Wv, Wg, Wo, window):
    assert int(window) == WINDOW
    x, ve, cos, sin, Wq, Wk, Wv, Wg, Wo = (
        np.asarray(a, dtype=np.float32)
        for a in (x, ve, cos, sin, Wq, Wk, Wv, Wg, Wo))
    assert x.shape == (B, T, C)
    from concourse.bass_utils import run_bass_kernel_spmd

    nc = _get_program()
    in_maps = make_in_maps(x, ve, cos, sin, Wq, Wk, Wv, Wg, Wo)
    res = run_bass_kernel_spmd(nc, in_maps, core_ids=list(range(N_CORES)))
    out = np.zeros((B, T, C), dtype=np.float32)
    for core in range(N_CORES):
        b = core // N_KV
        out[b] += np.asarray(res.results[core]["out"], dtype=np.float32)
    return out
